# revision 15
# baseline (speedup 1.0000x reference)
"""Trainium2 Bass kernel for nn_InterleavedHiddenMarkovChain_47261820125822.

Math: in the reference, the dense (N,N) score matrix M (N = S*S*K = 4608)
is -inf except where the full state tuple of x_old equals x_new's (the
`same` mask compares all K components), so each column has exactly K=2
finite entries and the scan collapses exactly.  With probability-domain
tables F_c[s,a] = softmax(choice)[c] * softmax(trans[c,s,:])[s] *
softmax(emis[c,s,:])[a] and u_c = ln F_c:

    beta[s0,s1] = p0[s0] + p1[s1] + sum_t LSE(u0[s0,y_t], u1[s1,y_t])
    answer      = LSE_{s0,s1} beta

Using LSE(a,b) = b + log1p(exp(a-b)) and grouping the t-sum by symbol
counts cnt[a] = #{t: y_t = a} (integer prep on host):

    beta = p0[s0] + (p1 + sum_a cnt[a] u1[s1,a])[s1]
         + sum_a cnt[a] * log1p(F0[s0,a] / F1[s1,a])

so the only big work is one (A=64 part, S*S=2304 free) elementwise
multiply F0[a,s0] * (1/F1)[a,s1] (both free-dim broadcasts), one
log1p activation, and a cnt-weighted column-sum matmul (fp32r).

Perf structure (v2; baseline 65.7us -> v1 34.9us):
 - ALL inputs ride ONE packed DRAM image [128, 387] (one DMA) with pad
   rows prefilled; v1 spent ~6us issuing 11 DMAs + transfer stalls.
 - One manual InstLoadActFuncSet of the joint exp+ln table (id 6) ->
   exactly one activation-table load (the greedy inserter otherwise
   reloads on every exp<->ln switch; 13x1283ns in the baseline).
 - A dummy gpsimd custom op up front hoists the GPSIMD ucode library
   load off the critical path (v1 stalled ~2.5us before the first
   partition_broadcast).
 - softmaxes by division (vector.reciprocal + fused accum_out row
   sums); trans diagonal extracted on-device by mask-multiply-reduce.
 - cnt-weighted t-reduction as fp32r PE matmuls (4x fp32 rate) over 3
   rotating PSUM banks; copies split vector-first so the PE never
   waits on a bank.
 - bias grid B built by PE outer product, folded in a (16,144) layout;
   final LSE on 16 partitions via gpsimd partition_all_reduce (the
   baseline burned ~20us in (1,2304) single-lane ops here).

Sharding across the 8 cores: the collapsed problem is ~150K flops, far
below per-core fixed overheads, so the sharding-hint's row-sharded psum
scheme would be pure loss.  We replicate: all 8 cores run the identical
NEFF SPMD, and the host takes core 0's scalar.  Host does only integer
index prep (symbol counts, diag masks, layout packing); all float math
is on-device.
"""

import numpy as np

import concourse.bass as bass
import concourse.bacc as bacc
import concourse.mybir as mybir
from concourse import tile
from concourse.bass_isa import ReduceOp
from concourse.bass_utils import run_bass_kernel_spmd

F32 = mybir.dt.float32
F32R = mybir.dt.float32r
BF16 = mybir.dt.bfloat16
AF = mybir.ActivationFunctionType
AX = mybir.AxisListType
OP = mybir.AluOpType

K, S, A, T = 2, 48, 64, 64
N2 = S * S          # 2304
N_CORES = 8
NEG = -30.0         # pad fill; exp(NEG - max) == 0 to fp32
# packed input column layout
C_TR, C_EM, C_IM, C_DM, C_P0, C_P1, C_CH, C_CNT = (
    0, 48, 112, 240, 288, 336, 384, 386)
PACKW = 387
# act_info.json table index of natural_log_exp_and_others (exp + ln in
# one piecewise-poly table -> a single ACT_TABLE_LOAD serves the kernel)
ACT_TABLE_EXP_LN = 6

_CACHED_NC = None


def _build_nc(dbg=False):
    nc = bacc.Bacc("TRN2", target_bir_lowering=False, debug=False)

    pk = nc.dram_tensor("packed", [128, PACKW], F32, kind="ExternalInput")
    out_d = nc.dram_tensor("out", [1, 1], F32, kind="ExternalOutput")
    if dbg:
        dbg_ept = nc.dram_tensor("dbg_ept", [A, 128], F32, kind="ExternalOutput")
        dbg_h1 = nc.dram_tensor("dbg_h1", [A, S], F32, kind="ExternalOutput")
        dbg_lf1 = nc.dram_tensor("dbg_lf1", [A, S], F32, kind="ExternalOutput")
        dbg_b2s = nc.dram_tensor("dbg_b2s", [S, S], F32, kind="ExternalOutput")
        dbg_nls = nc.dram_tensor("dbg_nls", [1, N2], F32, kind="ExternalOutput")
        dbg_cd = nc.dram_tensor("dbg_cd", [128, 1], F32, kind="ExternalOutput")

    with tile.TileContext(nc) as tc:
        with (
            tc.tile_pool(name="sb", bufs=1) as sb,
            tc.tile_pool(name="ps", bufs=1, space="PSUM") as ps,
        ):
            # activation table (exp+ln) + gpsimd ucode library warm-up;
            # both run while the input DMA is in flight
            nc.scalar.add_instruction(mybir.InstLoadActFuncSet(
                name=nc.get_next_instruction_name(),
                act_func_set_id=ACT_TABLE_EXP_LN, ins=[], outs=[]))
            ONE1 = sb.tile([1, 1], F32, tag="ONE1")
            nc.vector.memset(ONE1[:], 1.0)
            DUM = sb.tile([2, 1], F32, tag="DUM")
            nc.gpsimd.partition_broadcast(DUM[:], ONE1[:])

            BIG = sb.tile([128, PACKW], F32, tag="BIG")
            nc.sync.dma_start(BIG[:, 0:C_IM], pk[:, 0:C_IM])
            nc.scalar.dma_start(BIG[:, C_IM:C_DM], pk[:, C_IM:C_DM])
            nc.gpsimd.dma_start(BIG[:, C_DM:PACKW], pk[:, C_DM:PACKW])
            TRXv = BIG[:, C_TR:C_TR + S]
            EMXv = BIG[:, C_EM:C_EM + A]
            IMv = BIG[:, C_IM:C_IM + 128]
            DMv = BIG[:, C_DM:C_DM + S]
            PR0v = BIG[0:1, C_P0:C_P0 + S]
            PR1v = BIG[0:1, C_P1:C_P1 + S]
            CHXv = BIG[0:1, C_CH:C_CH + K]
            CNTv = BIG[0:A, C_CNT:C_CNT + 1]

            # constants
            ONES48 = sb.tile([1, S], F32, tag="ONES48")
            nc.vector.memset(ONES48[:], 1.0)
            O128 = sb.tile([1, 128], BF16, tag="O128")
            nc.vector.memset(O128[:], 1.0)
            cd = sb.tile([128, 1], F32, tag="cd")
            nc.vector.memset(cd[:], 0.0)

            # ---- row maxes (all only need BIG) ----
            def rmax(v, p, name):
                t = sb.tile([p, 1], F32, tag=f"nm{name}")
                nc.vector.tensor_reduce(t[:], v, axis=AX.X, op=OP.max,
                                        negate=True)
                return t

            nmE = rmax(EMXv, 128, "E")
            nmT = rmax(TRXv, 128, "T")
            nmC = rmax(CHXv, 1, "C")
            nmP0 = rmax(PR0v, 1, "P0")
            nmP1 = rmax(PR1v, 1, "P1")

            # ---- exps with fused row sums (scalar) ----
            def rexp(v, nm, p, w, name):
                e = sb.tile([p, w], F32, tag=f"e{name}")
                z = sb.tile([p, 1], F32, tag=f"z{name}")
                nc.scalar.activation(e[:], v, AF.Exp, bias=nm[:],
                                     accum_out=z[:])
                return e, z

            def rexp_na(v, nm, p, w, name, dt=F32):
                e = sb.tile([p, w], dt, tag=f"e{name}")
                nc.scalar.activation(e[:], v, AF.Exp, bias=nm[:])
                z = sb.tile([p, 1], F32, tag=f"z{name}")
                nc.vector.tensor_reduce(z[:], e[:], axis=AX.X, op=OP.add)
                return e, z

            EMe, ZE = rexp_na(EMXv, nmE, 128, A, "E", dt=BF16)
            TJ, ZT = rexp_na(TRXv, nmT, 128, S, "T")
            CHe, ZC = rexp_na(CHXv, nmC, 1, K, "C")
            P0e, ZP0 = rexp(PR0v, nmP0, 1, S, "P0")
            P1e, ZP1 = rexp(PR1v, nmP1, 1, S, "P1")
            lz0 = sb.tile([1, 1], F32, tag="lz0")
            nc.scalar.activation(lz0[:], ZP0[:], AF.Ln)
            lz1 = sb.tile([1, 1], F32, tag="lz1")
            nc.scalar.activation(lz1[:], ZP1[:], AF.Ln)

            # ---- vector normalize chain ----
            # emission row-normalizer 1/ZE folds into the diagonal scale
            ZEr = sb.tile([128, 1], F32, tag="ZEr")
            nc.vector.reciprocal(ZEr[:], ZE[:])
            ZTr0 = sb.tile([128, 1], F32, tag="ZTr0")
            nc.vector.reciprocal(ZTr0[:], ZT[:])
            ZTr = sb.tile([128, 1], F32, tag="ZTr")
            nc.vector.tensor_mul(ZTr[:], ZTr0[:], ZEr[:])
            # diagonal of exp(trans row): mask-multiply-reduce
            DJ = sb.tile([128, S], F32, tag="DJ")
            dex = sb.tile([128, 1], F32, tag="dex")
            nc.vector.scalar_tensor_tensor(DJ[:], TJ[:], 1.0, DMv,
                                           op0=OP.mult, op1=OP.mult,
                                           accum_out=dex[:])
            dpr = sb.tile([128, 1], F32, tag="dpr")
            nc.vector.tensor_mul(dpr[:], dex[:], ZTr[:])
            ZCr = sb.tile([1, 1], F32, tag="ZCr")
            nc.vector.reciprocal(ZCr[:], ZC[:])
            CPr = sb.tile([1, K], BF16, tag="CPr")
            nc.vector.tensor_scalar_mul(CPr[:], CHe[:], ZCr[:])

            # choice probs to all partitions via PE outer product
            CB_p = ps.tile([128, K], F32, tag="ps_cb")
            nc.tensor.matmul(CB_p[:], O128[:], CPr[:], start=True, stop=True)

            # cd[(c,s)] = c_prob[c] * d_prob[(c,s)]; scaled diagonal D128
            nc.vector.tensor_mul(cd[0:S, :], dpr[0:S, :], CB_p[0:S, 0:1])
            nc.vector.tensor_mul(cd[64:64 + S, :], dpr[64:64 + S, :],
                                 CB_p[64:64 + S, 1:2])
            D128 = sb.tile([128, 128], BF16, tag="D128")
            nc.vector.tensor_mul(D128[:], IMv,
                                 cd[:].broadcast_to([128, 128]))

            # EPT[a, (c,s)] = F_c(s, a)
            EPT_p = ps.tile([A, 128], F32, tag="ps_ept")
            nc.tensor.matmul(EPT_p[:], EMe[:], D128[:], start=True, stop=True)
            # ln F1 table for the R1 term (reads PSUM directly)
            LF1 = sb.tile([A, S], F32, tag="LF1")
            nc.scalar.activation(LF1[:], EPT_p[:, 64:64 + S], AF.Ln)
            # F0 -> SBUF; 1/F1 -> SBUF
            G0 = sb.tile([A, S], F32, tag="G0")
            nc.vector.tensor_copy(G0[:], EPT_p[:, 0:S])
            H1T = sb.tile([A, S], F32, tag="H1T")
            nc.vector.reciprocal(H1T[:], EPT_p[:, 64:64 + S])

            # cnt rounded to fp32r for the weighted-sum matmuls
            CNTR = sb.tile([A, 1], F32R, tag="CNTR")
            nc.gpsimd.tensor_copy(CNTR[:], CNTv)

            # ---- big phase: W[a, s0, s1] = F0[s0,a] / F1[s1,a] ----
            W = sb.tile([A, S, S], F32, tag="W")
            SPX = sb.tile([A, N2], F32R, tag="SPX")
            NCH = 3
            SCH = S // NCH
            for j in range(NCH):
                lo = j * SCH
                nc.vector.tensor_mul(
                    W[:, lo:lo + SCH, :],
                    G0[:, lo:lo + SCH].unsqueeze(2)
                      .broadcast_to([A, SCH, S]),
                    H1T[:].unsqueeze(1).broadcast_to([A, SCH, S]))
                nc.scalar.activation(
                    SPX[:, lo * S:(lo + SCH) * S],
                    W[:, lo:lo + SCH, :].rearrange("p a b -> p (a b)"),
                    AF.Ln, bias=1.0)

            # NL row: cnt-weighted column sums; 3 rotating psum banks,
            # copies split vector-first so the PE never waits
            NLS = sb.tile([1, N2], F32, tag="NLS")
            NL_p0 = ps.tile([1, 512], F32, tag="ps_nla")
            NL_p1 = ps.tile([1, 512], F32, tag="ps_nlb")
            NL_p2 = ps.tile([1, 512], F32, tag="ps_nlc")
            banks = [NL_p0, NL_p1, NL_p2]

            def nl_mm(c):
                lo = 512 * c
                w = min(512, N2 - lo)
                nc.tensor.matmul(banks[c % 3][:, 0:w], CNTR[:],
                                 SPX[:, lo:lo + w], start=True, stop=True)

            nl_mm(0)
            nl_mm(1)
            nl_mm(2)
            nc.vector.tensor_copy(NLS[:, 0:512], NL_p0[:])
            nl_mm(3)
            nc.vector.tensor_copy(NLS[:, 512:1024], NL_p1[:])
            nl_mm(4)
            nc.vector.tensor_copy(NLS[:, 1024:1536], NL_p2[:])
            nc.scalar.copy(NLS[:, 1536:2048], NL_p0[:])
            nc.scalar.copy(NLS[:, 2048:2304], NL_p1[:, 0:256])

            # ---- bias grid B[s0,s1] = p0[s0] + p1[s1] + R1[s1] ----
            def prow(src, nm, lz, name):
                row = sb.tile([1, S], F32, tag=f"row{name}")
                nc.vector.scalar_tensor_tensor(
                    row[:], src, nm[:], lz[:].broadcast_to([1, S]),
                    op0=OP.add, op1=OP.subtract)
                return row

            p0row = prow(PR0v, nmP0, lz0, "P0")
            p1row = prow(PR1v, nmP1, lz1, "P1")
            R1_p = ps.tile([1, S], F32, tag="ps_r1")
            nc.tensor.matmul(R1_p[:], CNTv, LF1[:], start=True, stop=True)
            q1 = sb.tile([1, S], F32, tag="q1")
            nc.vector.tensor_add(q1[:], p1row[:], R1_p[:])
            p0c_p = ps.tile([S, 1], F32, tag="ps_p0")
            nc.tensor.matmul(p0c_p[:], p0row[:], ONE1[:], start=True,
                             stop=True)
            p0c = sb.tile([S, 1], F32, tag="p0c")
            nc.vector.tensor_copy(p0c[:], p0c_p[:])
            B2d_p = ps.tile([S, S], F32, tag="ps_b2d")
            nc.tensor.matmul(B2d_p[:], ONES48[:], q1[:], start=True,
                             stop=True)
            B2s = sb.tile([S, S], F32, tag="B2s")
            nc.vector.tensor_scalar_add(B2s[:], B2d_p[:], p0c[:])
            B16 = sb.tile([16, 144], F32, tag="B16")
            nc.scalar.dma_start(B16[:], B2s[:])

            NL16 = sb.tile([16, 144], F32, tag="NL16")
            nc.sync.dma_start(NL16[:], NLS[:])

            # ---- tail: LSE over beta = NL + B on 16 partitions ----
            T16 = sb.tile([16, 144], F32, tag="T16")
            nc.vector.tensor_add(T16[:], NL16[:], B16[:])
            M1 = sb.tile([16, 1], F32, tag="M1")
            nc.vector.tensor_reduce(M1[:], T16[:], axis=AX.X, op=OP.max)
            Mg = sb.tile([16, 1], F32, tag="Mg")
            nc.gpsimd.partition_all_reduce(Mg[:], M1[:], 16, ReduceOp.max)
            Mn = sb.tile([16, 1], F32, tag="Mn")
            nc.vector.tensor_scalar_mul(Mn[:], Mg[:], -1.0)
            EX16 = sb.tile([16, 144], F32, tag="EX16")
            S1 = sb.tile([16, 1], F32, tag="S1")
            nc.scalar.activation(EX16[:], T16[:], AF.Exp, bias=Mn[:],
                                 accum_out=S1[:])
            Sg = sb.tile([16, 1], F32, tag="Sg")
            nc.gpsimd.partition_all_reduce(Sg[:], S1[:], 16, ReduceOp.add)
            lnS = sb.tile([1, 1], F32, tag="lnS")
            nc.scalar.activation(lnS[:], Sg[0:1, :], AF.Ln)
            ans = sb.tile([1, 1], F32, tag="ans")
            nc.vector.tensor_add(ans[:], lnS[:], Mg[0:1, :])
            nc.sync.dma_start(out_d[:, :], ans[:])
            if dbg:
                EPTs = sb.tile([A, 128], F32, tag="EPTs")
                nc.vector.tensor_copy(EPTs[:], EPT_p[:])
                nc.sync.dma_start(dbg_ept[:, :], EPTs[:])
                nc.sync.dma_start(dbg_h1[:, :], H1T[:])
                nc.sync.dma_start(dbg_lf1[:, :], LF1[:])
                nc.sync.dma_start(dbg_b2s[:, :], B2s[:])
                nc.sync.dma_start(dbg_nls[:, :], NLS[:])
                nc.sync.dma_start(dbg_cd[:, :], cd[:])

    nc.compile()
    return nc


def _host_inputs(ys, transition, emission, choice, prior):
    ys = np.asarray(ys).astype(np.int64)
    packed = np.zeros((128, PACKW), np.float32)
    tr = np.asarray(transition, np.float32)
    em = np.asarray(emission, np.float32)
    pri = np.asarray(prior, np.float32)
    for c in range(K):
        r = c * 64
        packed[r:r + S, C_TR:C_TR + S] = tr[c]
        packed[r:r + S, C_EM:C_EM + A] = em[c]
        # pads: exp(NEG - max) == 0 keeps row sums finite
        packed[r + S:r + 64, C_TR:C_TR + S] = NEG
        packed[r + S:r + 64, C_EM:C_EM + A] = NEG
        for s in range(S):
            packed[r + s, C_IM + r + s] = 1.0   # scaled-diag mask
            packed[r + s, C_DM + s] = 1.0       # trans-diag extract mask
    packed[0, C_P0:C_P0 + S] = pri[0]
    packed[0, C_P1:C_P1 + S] = pri[1]
    packed[0, C_CH:C_CH + K] = np.asarray(choice, np.float32)
    packed[0:A, C_CNT] = np.bincount(ys, minlength=A).astype(np.float32)
    return {"packed": packed}


def kernel(ys, transition, emission, choice, prior):
    global _CACHED_NC
    if _CACHED_NC is None:
        _CACHED_NC = _build_nc()
    in_map = _host_inputs(ys, transition, emission, choice, prior)
    in_maps = [dict(in_map) for _ in range(N_CORES)]
    res = run_bass_kernel_spmd(_CACHED_NC, in_maps,
                               core_ids=list(range(N_CORES)))
    return np.float32(res.results[0]["out"][0, 0]).reshape(())


# revision 18
# speedup vs baseline: 1.1555x; 1.1555x over previous
"""Trainium2 Bass kernel for nn_InterleavedHiddenMarkovChain_47261820125822.

Math: in the reference, the dense (N,N) score matrix M (N = S*S*K = 4608)
is -inf except where the full state tuple of x_old equals x_new's (the
`same` mask compares all K components), so each column has exactly K=2
finite entries and the scan collapses exactly.  With probability-domain
tables F_c[s,a] = softmax(choice)[c] * softmax(trans[c,s,:])[s] *
softmax(emis[c,s,:])[a] and u_c = ln F_c:

    beta[s0,s1] = p0[s0] + p1[s1] + sum_t LSE(u0[s0,y_t], u1[s1,y_t])
    answer      = LSE_{s0,s1} beta

Using LSE(a,b) = b + log1p(exp(a-b)) and grouping the t-sum by symbol
counts cnt[a] = #{t: y_t = a} (integer prep on host):

    beta = p0[s0] + (p1 + sum_a cnt[a] u1[s1,a])[s1]
         + sum_a cnt[a] * log1p(F0[s0,a] / F1[s1,a])

so the only big work is one (A=64 part, S*S=2304 free) elementwise
multiply F0[a,s0] * (1/F1)[a,s1] (both free-dim broadcasts), one
log1p activation, and a cnt-weighted column-sum matmul (fp32r).

Perf structure (v2; baseline 65.7us -> v1 34.9us):
 - ALL inputs ride ONE packed DRAM image [128, 387] (one DMA) with pad
   rows prefilled; v1 spent ~6us issuing 11 DMAs + transfer stalls.
 - One manual InstLoadActFuncSet of the joint exp+ln table (id 6) ->
   exactly one activation-table load (the greedy inserter otherwise
   reloads on every exp<->ln switch; 13x1283ns in the baseline).
 - A dummy gpsimd custom op up front hoists the GPSIMD ucode library
   load off the critical path (v1 stalled ~2.5us before the first
   partition_broadcast).
 - softmaxes by division (vector.reciprocal + fused accum_out row
   sums); trans diagonal extracted on-device by mask-multiply-reduce.
 - cnt-weighted t-reduction as fp32r PE matmuls (4x fp32 rate) over 3
   rotating PSUM banks; copies split vector-first so the PE never
   waits on a bank.
 - bias grid B built by PE outer product, folded in a (16,144) layout;
   final LSE on 16 partitions via gpsimd partition_all_reduce (the
   baseline burned ~20us in (1,2304) single-lane ops here).

Sharding across the 8 cores: the collapsed problem is ~150K flops, far
below per-core fixed overheads, so the sharding-hint's row-sharded psum
scheme would be pure loss.  We replicate: all 8 cores run the identical
NEFF SPMD, and the host takes core 0's scalar.  Host does only integer
index prep (symbol counts, diag masks, layout packing); all float math
is on-device.
"""

import numpy as np

import concourse.bass as bass
import concourse.bacc as bacc
import concourse.mybir as mybir
from concourse import tile
from concourse.bass_isa import ReduceOp
from concourse.bass_utils import run_bass_kernel_spmd

F32 = mybir.dt.float32
F32R = mybir.dt.float32r
BF16 = mybir.dt.bfloat16
AF = mybir.ActivationFunctionType
AX = mybir.AxisListType
OP = mybir.AluOpType

K, S, A, T = 2, 48, 64, 64
N2 = S * S          # 2304
N_CORES = 8
NEG = -30.0         # pad fill; exp(NEG - max) == 0 to fp32
# packed input column layout
C_TR, C_EM, C_DM, C_P0, C_P1, C_CH, C_CNT = (
    0, 48, 112, 160, 208, 256, 258)
PACKW = 259
# act_info.json table index of natural_log_exp_and_others (exp + ln in
# one piecewise-poly table -> a single ACT_TABLE_LOAD serves the kernel)
ACT_TABLE_EXP_LN = 6

_CACHED_NC = None


def _build_nc(dbg=False):
    nc = bacc.Bacc("TRN2", target_bir_lowering=False, debug=False)

    pk = nc.dram_tensor("packed", [128, PACKW], F32, kind="ExternalInput")
    im_d = nc.dram_tensor("imask", [128, 128], BF16, kind="ExternalInput")
    out_d = nc.dram_tensor("out", [1, 1], F32, kind="ExternalOutput")
    if dbg:
        dbg_ept = nc.dram_tensor("dbg_ept", [A, 128], F32, kind="ExternalOutput")
        dbg_h1 = nc.dram_tensor("dbg_h1", [A, S], F32, kind="ExternalOutput")
        dbg_lf1 = nc.dram_tensor("dbg_lf1", [A, S], F32, kind="ExternalOutput")
        dbg_b2s = nc.dram_tensor("dbg_b2s", [S, S], F32, kind="ExternalOutput")
        dbg_nls = nc.dram_tensor("dbg_nls", [1, N2], F32, kind="ExternalOutput")
        dbg_cd = nc.dram_tensor("dbg_cd", [128, 1], F32, kind="ExternalOutput")

    with tile.TileContext(nc) as tc:
        with (
            tc.tile_pool(name="sb", bufs=1) as sb,
            tc.tile_pool(name="ps", bufs=1, space="PSUM") as ps,
        ):
            # activation table (exp+ln) + gpsimd ucode library warm-up;
            # both run while the input DMA is in flight
            nc.scalar.add_instruction(mybir.InstLoadActFuncSet(
                name=nc.get_next_instruction_name(),
                act_func_set_id=ACT_TABLE_EXP_LN, ins=[], outs=[]))
            ONE1 = sb.tile([1, 1], F32, tag="ONE1")
            nc.vector.memset(ONE1[:], 1.0)
            DUM = sb.tile([2, 1], F32, tag="DUM")
            nc.gpsimd.partition_broadcast(DUM[:], ONE1[:])

            BIG = sb.tile([128, PACKW], F32, tag="BIG")
            IM = sb.tile([128, 128], BF16, tag="IM")
            nc.sync.dma_start(BIG[:, 0:C_DM], pk[:, 0:C_DM])
            nc.scalar.dma_start(BIG[:, C_DM:PACKW], pk[:, C_DM:PACKW])
            nc.gpsimd.dma_start(IM[:], im_d[:, :])
            TRXv = BIG[:, C_TR:C_TR + S]
            EMXv = BIG[:, C_EM:C_EM + A]
            DMv = BIG[:, C_DM:C_DM + S]
            PR0v = BIG[0:1, C_P0:C_P0 + S]
            PR1v = BIG[0:1, C_P1:C_P1 + S]
            CHXv = BIG[0:1, C_CH:C_CH + K]
            CNTv = BIG[0:A, C_CNT:C_CNT + 1]

            # constants
            ONES48 = sb.tile([1, S], F32, tag="ONES48")
            nc.vector.memset(ONES48[:], 1.0)
            O128 = sb.tile([1, 128], BF16, tag="O128")
            nc.vector.memset(O128[:], 1.0)
            cd = sb.tile([128, 1], F32, tag="cd")
            nc.vector.memset(cd[:], 0.0)

            # ---- row maxes (all only need BIG) ----
            def rmax(v, p, name):
                t = sb.tile([p, 1], F32, tag=f"nm{name}")
                nc.vector.tensor_reduce(t[:], v, axis=AX.X, op=OP.max,
                                        negate=True)
                return t

            nmE = rmax(EMXv, 128, "E")
            nmT = rmax(TRXv, 128, "T")
            nmC = rmax(CHXv, 1, "C")
            nmP0 = rmax(PR0v, 1, "P0")
            nmP1 = rmax(PR1v, 1, "P1")

            # ---- exps with fused row sums (scalar) ----
            def rexp(v, nm, p, w, name):
                e = sb.tile([p, w], F32, tag=f"e{name}")
                z = sb.tile([p, 1], F32, tag=f"z{name}")
                nc.scalar.activation(e[:], v, AF.Exp, bias=nm[:],
                                     accum_out=z[:])
                return e, z

            def rexp_na(v, nm, p, w, name, dt=F32):
                e = sb.tile([p, w], dt, tag=f"e{name}")
                nc.scalar.activation(e[:], v, AF.Exp, bias=nm[:])
                z = sb.tile([p, 1], F32, tag=f"z{name}")
                nc.vector.tensor_reduce(z[:], e[:], axis=AX.X, op=OP.add)
                return e, z

            EMe, ZE = rexp_na(EMXv, nmE, 128, A, "E", dt=BF16)
            TJ, ZT = rexp_na(TRXv, nmT, 128, S, "T")
            CHe, ZC = rexp_na(CHXv, nmC, 1, K, "C")
            P0e, ZP0 = rexp(PR0v, nmP0, 1, S, "P0")
            P1e, ZP1 = rexp(PR1v, nmP1, 1, S, "P1")
            lz0 = sb.tile([1, 1], F32, tag="lz0")
            nc.scalar.activation(lz0[:], ZP0[:], AF.Ln)
            lz1 = sb.tile([1, 1], F32, tag="lz1")
            nc.scalar.activation(lz1[:], ZP1[:], AF.Ln)

            # ---- vector normalize chain ----
            # emission row-normalizer 1/ZE folds into the diagonal scale
            ZEr = sb.tile([128, 1], F32, tag="ZEr")
            nc.vector.reciprocal(ZEr[:], ZE[:])
            ZTr0 = sb.tile([128, 1], F32, tag="ZTr0")
            nc.vector.reciprocal(ZTr0[:], ZT[:])
            ZTr = sb.tile([128, 1], F32, tag="ZTr")
            nc.vector.tensor_mul(ZTr[:], ZTr0[:], ZEr[:])
            # diagonal of exp(trans row): mask-multiply-reduce
            DJ = sb.tile([128, S], F32, tag="DJ")
            dex = sb.tile([128, 1], F32, tag="dex")
            nc.vector.scalar_tensor_tensor(DJ[:], TJ[:], 1.0, DMv,
                                           op0=OP.mult, op1=OP.mult,
                                           accum_out=dex[:])
            dpr = sb.tile([128, 1], F32, tag="dpr")
            nc.scalar.activation(dpr[:], dex[:], AF.Copy, bias=0.0,
                                 scale=ZTr[:])
            ZCr = sb.tile([1, 1], F32, tag="ZCr")
            nc.vector.reciprocal(ZCr[:], ZC[:])
            CPr = sb.tile([1, K], BF16, tag="CPr")
            nc.scalar.activation(CPr[:], CHe[:], AF.Copy, bias=0.0,
                                 scale=ZCr[:])

            # choice probs to all partitions via PE outer product
            CB_p = ps.tile([128, K], F32, tag="ps_cb")
            nc.tensor.matmul(CB_p[:], O128[:], CPr[:], start=True, stop=True)

            # cd[(c,s)] = c_prob[c] * d_prob[(c,s)]; scaled diagonal D128
            nc.vector.tensor_mul(cd[0:S, :], dpr[0:S, :], CB_p[0:S, 0:1])
            nc.vector.tensor_mul(cd[64:64 + S, :], dpr[64:64 + S, :],
                                 CB_p[64:64 + S, 1:2])
            D128 = sb.tile([128, 128], BF16, tag="D128")
            nc.scalar.activation(D128[:], IM[:], AF.Copy, bias=0.0,
                                 scale=cd[:])

            # EPT[a, (c,s)] = F_c(s, a)
            EPT_p = ps.tile([A, 128], F32, tag="ps_ept")
            nc.tensor.matmul(EPT_p[:], EMe[:], D128[:], start=True, stop=True)
            # ln F1 table for the R1 term (reads PSUM directly)
            LF1 = sb.tile([A, S], F32, tag="LF1")
            nc.scalar.activation(LF1[:], EPT_p[:, 64:64 + S], AF.Ln)
            # F0 -> SBUF; 1/F1 -> SBUF
            G0 = sb.tile([A, S], F32, tag="G0")
            nc.vector.tensor_copy(G0[:], EPT_p[:, 0:S])
            H1T = sb.tile([A, S], F32, tag="H1T")
            nc.vector.reciprocal(H1T[:], EPT_p[:, 64:64 + S])

            # cnt rounded to fp32r for the weighted-sum matmuls
            CNTR = sb.tile([A, 1], F32R, tag="CNTR")
            nc.gpsimd.tensor_copy(CNTR[:], CNTv)

            # ---- big phase: W[a, s0, s1] = F0[s0,a] / F1[s1,a] ----
            W = sb.tile([A, S, S], F32, tag="W")
            SPX = sb.tile([A, N2], F32R, tag="SPX")
            NCH = 3
            SCH = S // NCH
            for j in range(NCH):
                lo = j * SCH
                nc.vector.tensor_mul(
                    W[:, lo:lo + SCH, :],
                    G0[:, lo:lo + SCH].unsqueeze(2)
                      .broadcast_to([A, SCH, S]),
                    H1T[:].unsqueeze(1).broadcast_to([A, SCH, S]))
                nc.scalar.activation(
                    SPX[:, lo * S:(lo + SCH) * S],
                    W[:, lo:lo + SCH, :].rearrange("p a b -> p (a b)"),
                    AF.Ln, bias=1.0)

            # NL row: cnt-weighted column sums; 3 rotating psum banks,
            # copies split vector-first so the PE never waits
            NLS = sb.tile([1, N2], F32, tag="NLS")
            NL_p0 = ps.tile([1, 512], F32, tag="ps_nla")
            NL_p1 = ps.tile([1, 512], F32, tag="ps_nlb")
            NL_p2 = ps.tile([1, 512], F32, tag="ps_nlc")
            banks = [NL_p0, NL_p1, NL_p2]

            def nl_mm(c):
                lo = 512 * c
                w = min(512, N2 - lo)
                nc.tensor.matmul(banks[c % 3][:, 0:w], CNTR[:],
                                 SPX[:, lo:lo + w], start=True, stop=True)

            nl_mm(0)
            nl_mm(1)
            nl_mm(2)
            nc.vector.tensor_copy(NLS[:, 0:512], NL_p0[:])
            nl_mm(3)
            nc.vector.tensor_copy(NLS[:, 512:1024], NL_p1[:])
            nl_mm(4)
            nc.vector.tensor_copy(NLS[:, 1024:1536], NL_p2[:])
            nc.scalar.copy(NLS[:, 1536:2048], NL_p0[:])
            nc.scalar.copy(NLS[:, 2048:2304], NL_p1[:, 0:256])

            # ---- bias grid B[s0,s1] = p0[s0] + p1[s1] + R1[s1] ----
            def prow(src, nm, lz, name):
                row = sb.tile([1, S], F32, tag=f"row{name}")
                nc.vector.scalar_tensor_tensor(
                    row[:], src, nm[:], lz[:].broadcast_to([1, S]),
                    op0=OP.add, op1=OP.subtract)
                return row

            p0row = prow(PR0v, nmP0, lz0, "P0")
            p1row = prow(PR1v, nmP1, lz1, "P1")
            R1_p = ps.tile([1, S], F32, tag="ps_r1")
            nc.tensor.matmul(R1_p[:], CNTv, LF1[:], start=True, stop=True)
            q1 = sb.tile([1, S], F32, tag="q1")
            nc.vector.tensor_add(q1[:], p1row[:], R1_p[:])
            p0c_p = ps.tile([S, 1], F32, tag="ps_p0")
            nc.tensor.matmul(p0c_p[:], p0row[:], ONE1[:], start=True,
                             stop=True)
            p0c = sb.tile([S, 1], F32, tag="p0c")
            nc.vector.tensor_copy(p0c[:], p0c_p[:])
            B2d_p = ps.tile([S, S], F32, tag="ps_b2d")
            nc.tensor.matmul(B2d_p[:], ONES48[:], q1[:], start=True,
                             stop=True)
            B2s = sb.tile([S, S], F32, tag="B2s")
            nc.vector.tensor_scalar_add(B2s[:], B2d_p[:], p0c[:])
            B16 = sb.tile([16, 144], F32, tag="B16")
            nc.scalar.dma_start(B16[:], B2s[:])

            NL16 = sb.tile([16, 144], F32, tag="NL16")
            nc.sync.dma_start(NL16[:], NLS[:])

            # ---- tail: LSE over beta = NL + B on 16 partitions ----
            T16 = sb.tile([16, 144], F32, tag="T16")
            nc.vector.tensor_add(T16[:], NL16[:], B16[:])
            M1 = sb.tile([16, 1], F32, tag="M1")
            nc.vector.tensor_reduce(M1[:], T16[:], axis=AX.X, op=OP.max)
            Mg = sb.tile([16, 1], F32, tag="Mg")
            nc.gpsimd.partition_all_reduce(Mg[:], M1[:], 16, ReduceOp.max)
            Mn = sb.tile([16, 1], F32, tag="Mn")
            nc.vector.tensor_scalar_mul(Mn[:], Mg[:], -1.0)
            EX16 = sb.tile([16, 144], F32, tag="EX16")
            S1 = sb.tile([16, 1], F32, tag="S1")
            nc.scalar.activation(EX16[:], T16[:], AF.Exp, bias=Mn[:],
                                 accum_out=S1[:])
            O16 = sb.tile([16, 1], F32, tag="O16")
            nc.vector.memset(O16[:], 1.0)
            Sg_p = ps.tile([1, 1], F32, tag="ps_r1")
            nc.tensor.matmul(Sg_p[:], S1[:], O16[:], start=True, stop=True)
            lnS = sb.tile([1, 1], F32, tag="lnS")
            nc.scalar.activation(lnS[:], Sg_p[:], AF.Ln)
            ans = sb.tile([1, 1], F32, tag="ans")
            nc.vector.tensor_add(ans[:], lnS[:], Mg[0:1, :])
            nc.sync.dma_start(out_d[:, :], ans[:])
            if dbg:
                EPTs = sb.tile([A, 128], F32, tag="EPTs")
                nc.vector.tensor_copy(EPTs[:], EPT_p[:])
                nc.sync.dma_start(dbg_ept[:, :], EPTs[:])
                nc.sync.dma_start(dbg_h1[:, :], H1T[:])
                nc.sync.dma_start(dbg_lf1[:, :], LF1[:])
                nc.sync.dma_start(dbg_b2s[:, :], B2s[:])
                nc.sync.dma_start(dbg_nls[:, :], NLS[:])
                nc.sync.dma_start(dbg_cd[:, :], cd[:])

    nc.compile()
    return nc


def _host_inputs(ys, transition, emission, choice, prior):
    ys = np.asarray(ys).astype(np.int64)
    packed = np.zeros((128, PACKW), np.float32)
    tr = np.asarray(transition, np.float32)
    em = np.asarray(emission, np.float32)
    pri = np.asarray(prior, np.float32)
    for c in range(K):
        r = c * 64
        packed[r:r + S, C_TR:C_TR + S] = tr[c]
        packed[r:r + S, C_EM:C_EM + A] = em[c]
        # pads: exp(NEG - max) == 0 keeps row sums finite
        packed[r + S:r + 64, C_TR:C_TR + S] = NEG
        packed[r + S:r + 64, C_EM:C_EM + A] = NEG
        for s in range(S):
            packed[r + s, C_DM + s] = 1.0       # trans-diag extract mask
    packed[0, C_P0:C_P0 + S] = pri[0]
    packed[0, C_P1:C_P1 + S] = pri[1]
    packed[0, C_CH:C_CH + K] = np.asarray(choice, np.float32)
    packed[0:A, C_CNT] = np.bincount(ys, minlength=A).astype(np.float32)
    imask = np.zeros((128, 128), np.float32)
    for c in range(K):
        for s in range(S):
            imask[c * 64 + s, c * 64 + s] = 1.0
    import jax.numpy as jnp
    return {"packed": packed, "imask": imask.astype(jnp.bfloat16)}


def kernel(ys, transition, emission, choice, prior):
    global _CACHED_NC
    if _CACHED_NC is None:
        _CACHED_NC = _build_nc()
    in_map = _host_inputs(ys, transition, emission, choice, prior)
    in_maps = [dict(in_map) for _ in range(N_CORES)]
    res = run_bass_kernel_spmd(_CACHED_NC, in_maps,
                               core_ids=list(range(N_CORES)))
    return np.float32(res.results[0]["out"][0, 0]).reshape(())


# revision 19
# speedup vs baseline: 1.1901x; 1.0300x over previous
"""Trainium2 Bass kernel for nn_InterleavedHiddenMarkovChain_47261820125822.

Math: in the reference, the dense (N,N) score matrix M (N = S*S*K = 4608)
is -inf except where the full state tuple of x_old equals x_new's (the
`same` mask compares all K components), so each column has exactly K=2
finite entries and the scan collapses exactly.  With probability-domain
tables F_c[s,a] = softmax(choice)[c] * softmax(trans[c,s,:])[s] *
softmax(emis[c,s,:])[a] and u_c = ln F_c:

    beta[s0,s1] = p0[s0] + p1[s1] + sum_t LSE(u0[s0,y_t], u1[s1,y_t])
    answer      = LSE_{s0,s1} beta

Using LSE(a,b) = b + log1p(exp(a-b)) and grouping the t-sum by symbol
counts cnt[a] = #{t: y_t = a} (integer prep on host):

    beta = p0[s0] + (p1 + sum_a cnt[a] u1[s1,a])[s1]
         + sum_a cnt[a] * log1p(F0[s0,a] / F1[s1,a])

so the only big work is one (A=64 part, S*S=2304 free) elementwise
multiply F0[a,s0] * (1/F1)[a,s1] (both free-dim broadcasts), one
log1p activation, and a cnt-weighted column-sum matmul (fp32r).

Perf structure (v2; baseline 65.7us -> v1 34.9us):
 - ALL inputs ride ONE packed DRAM image [128, 387] (one DMA) with pad
   rows prefilled; v1 spent ~6us issuing 11 DMAs + transfer stalls.
 - One manual InstLoadActFuncSet of the joint exp+ln table (id 6) ->
   exactly one activation-table load (the greedy inserter otherwise
   reloads on every exp<->ln switch; 13x1283ns in the baseline).
 - A dummy gpsimd custom op up front hoists the GPSIMD ucode library
   load off the critical path (v1 stalled ~2.5us before the first
   partition_broadcast).
 - softmaxes by division (vector.reciprocal + fused accum_out row
   sums); trans diagonal extracted on-device by mask-multiply-reduce.
 - cnt-weighted t-reduction as fp32r PE matmuls (4x fp32 rate) over 3
   rotating PSUM banks; copies split vector-first so the PE never
   waits on a bank.
 - bias grid B built by PE outer product, folded in a (16,144) layout;
   final LSE on 16 partitions via gpsimd partition_all_reduce (the
   baseline burned ~20us in (1,2304) single-lane ops here).

Sharding across the 8 cores: the collapsed problem is ~150K flops, far
below per-core fixed overheads, so the sharding-hint's row-sharded psum
scheme would be pure loss.  We replicate: all 8 cores run the identical
NEFF SPMD, and the host takes core 0's scalar.  Host does only integer
index prep (symbol counts, diag masks, layout packing); all float math
is on-device.
"""

import numpy as np

import concourse.bass as bass
import concourse.bacc as bacc
import concourse.mybir as mybir
from concourse import tile
from concourse.bass_isa import ReduceOp
from concourse.bass_utils import run_bass_kernel_spmd

F32 = mybir.dt.float32
F32R = mybir.dt.float32r
BF16 = mybir.dt.bfloat16
AF = mybir.ActivationFunctionType
AX = mybir.AxisListType
OP = mybir.AluOpType

K, S, A, T = 2, 48, 64, 64
N2 = S * S          # 2304
N_CORES = 8
NEG = -30.0         # pad fill; exp(NEG - max) == 0 to fp32
# packed input column layout
C_TR, C_EM, C_DM, C_P0, C_P1, C_CH, C_CNT = (
    0, 48, 112, 160, 208, 256, 258)
PACKW = 259
# act_info.json table index of natural_log_exp_and_others (exp + ln in
# one piecewise-poly table -> a single ACT_TABLE_LOAD serves the kernel)
ACT_TABLE_EXP_LN = 6

_CACHED_NC = None


def _build_nc(dbg=False):
    nc = bacc.Bacc("TRN2", target_bir_lowering=False, debug=False)

    pk = nc.dram_tensor("packed", [128, PACKW], F32, kind="ExternalInput")
    im_d = nc.dram_tensor("imask", [128, 128], BF16, kind="ExternalInput")
    out_d = nc.dram_tensor("out", [1, 1], F32, kind="ExternalOutput")
    if dbg:
        dbg_ept = nc.dram_tensor("dbg_ept", [A, 128], F32, kind="ExternalOutput")
        dbg_h1 = nc.dram_tensor("dbg_h1", [A, S], F32, kind="ExternalOutput")
        dbg_lf1 = nc.dram_tensor("dbg_lf1", [A, S], F32, kind="ExternalOutput")
        dbg_b2s = nc.dram_tensor("dbg_b2s", [S, S], F32, kind="ExternalOutput")
        dbg_nls = nc.dram_tensor("dbg_nls", [1, N2], F32, kind="ExternalOutput")
        dbg_cd = nc.dram_tensor("dbg_cd", [128, 1], F32, kind="ExternalOutput")

    with tile.TileContext(nc) as tc:
        with (
            tc.tile_pool(name="sb", bufs=1) as sb,
            tc.tile_pool(name="ps", bufs=1, space="PSUM") as ps,
        ):
            # activation table (exp+ln) + gpsimd ucode library warm-up;
            # both run while the input DMA is in flight
            nc.scalar.add_instruction(mybir.InstLoadActFuncSet(
                name=nc.get_next_instruction_name(),
                act_func_set_id=ACT_TABLE_EXP_LN, ins=[], outs=[]))
            ONE1 = sb.tile([1, 1], F32, tag="ONE1")
            nc.vector.memset(ONE1[:], 1.0)
            DUM = sb.tile([2, 1], F32, tag="DUM")
            nc.gpsimd.partition_broadcast(DUM[:], ONE1[:])

            BIG = sb.tile([128, PACKW], F32, tag="BIG")
            IM = sb.tile([128, 128], BF16, tag="IM")
            nc.sync.dma_start(BIG[:, 0:C_DM], pk[:, 0:C_DM])
            nc.scalar.dma_start(BIG[:, C_DM:PACKW], pk[:, C_DM:PACKW])
            nc.gpsimd.dma_start(IM[:], im_d[:, :])
            TRXv = BIG[:, C_TR:C_TR + S]
            EMXv = BIG[:, C_EM:C_EM + A]
            DMv = BIG[:, C_DM:C_DM + S]
            PR0v = BIG[0:1, C_P0:C_P0 + S]
            PR1v = BIG[0:1, C_P1:C_P1 + S]
            CHXv = BIG[0:1, C_CH:C_CH + K]
            CNTv = BIG[0:A, C_CNT:C_CNT + 1]

            # constants
            ONES48 = sb.tile([1, S], F32, tag="ONES48")
            nc.vector.memset(ONES48[:], 1.0)
            O128 = sb.tile([1, 128], BF16, tag="O128")
            nc.vector.memset(O128[:], 1.0)
            cd = sb.tile([128, 1], F32, tag="cd")
            nc.vector.memset(cd[:], 0.0)

            # ---- row maxes (all only need BIG) ----
            def rmax(v, p, name):
                t = sb.tile([p, 1], F32, tag=f"nm{name}")
                nc.vector.tensor_reduce(t[:], v, axis=AX.X, op=OP.max,
                                        negate=True)
                return t

            nmE = rmax(EMXv, 128, "E")
            nmT = rmax(TRXv, 128, "T")
            nmC = rmax(CHXv, 1, "C")
            nmP0 = rmax(PR0v, 1, "P0")
            nmP1 = rmax(PR1v, 1, "P1")

            # ---- exps with fused row sums (scalar) ----
            def rexp(v, nm, p, w, name):
                e = sb.tile([p, w], F32, tag=f"e{name}")
                z = sb.tile([p, 1], F32, tag=f"z{name}")
                nc.scalar.activation(e[:], v, AF.Exp, bias=nm[:],
                                     accum_out=z[:])
                return e, z

            def rexp_na(v, nm, p, w, name, dt=F32):
                e = sb.tile([p, w], dt, tag=f"e{name}")
                nc.scalar.activation(e[:], v, AF.Exp, bias=nm[:])
                z = sb.tile([p, 1], F32, tag=f"z{name}")
                nc.vector.tensor_reduce(z[:], e[:], axis=AX.X, op=OP.add)
                return e, z

            EMe, ZE = rexp_na(EMXv, nmE, 128, A, "E", dt=BF16)
            TJ, ZT = rexp_na(TRXv, nmT, 128, S, "T")
            CHe, ZC = rexp_na(CHXv, nmC, 1, K, "C")

            # ---- vector normalize chain ----
            # emission row-normalizer 1/ZE folds into the diagonal scale
            ZEr = sb.tile([128, 1], F32, tag="ZEr")
            nc.vector.reciprocal(ZEr[:], ZE[:])
            ZTr0 = sb.tile([128, 1], F32, tag="ZTr0")
            nc.vector.reciprocal(ZTr0[:], ZT[:])
            ZTr = sb.tile([128, 1], F32, tag="ZTr")
            nc.vector.tensor_mul(ZTr[:], ZTr0[:], ZEr[:])
            # diagonal of exp(trans row): mask-multiply-reduce
            DJ = sb.tile([128, S], F32, tag="DJ")
            dex = sb.tile([128, 1], F32, tag="dex")
            nc.vector.scalar_tensor_tensor(DJ[:], TJ[:], 1.0, DMv,
                                           op0=OP.mult, op1=OP.mult,
                                           accum_out=dex[:])
            dpr = sb.tile([128, 1], F32, tag="dpr")
            nc.vector.tensor_mul(dpr[:], dex[:], ZTr[:])
            ZCr = sb.tile([1, 1], F32, tag="ZCr")
            nc.vector.reciprocal(ZCr[:], ZC[:])
            CPr = sb.tile([1, K], BF16, tag="CPr")
            nc.scalar.activation(CPr[:], CHe[:], AF.Copy, bias=0.0,
                                 scale=ZCr[:])

            # choice probs to all partitions via PE outer product
            CB_p = ps.tile([128, K], F32, tag="ps_cb")
            nc.tensor.matmul(CB_p[:], O128[:], CPr[:], start=True, stop=True)

            # cd[(c,s)] = c_prob[c] * d_prob[(c,s)]; scaled diagonal D128
            nc.vector.tensor_mul(cd[0:S, :], dpr[0:S, :], CB_p[0:S, 0:1])
            nc.vector.tensor_mul(cd[64:64 + S, :], dpr[64:64 + S, :],
                                 CB_p[64:64 + S, 1:2])
            D128 = sb.tile([128, 128], BF16, tag="D128")
            nc.vector.tensor_mul(D128[:], IM[:],
                                 cd[:].broadcast_to([128, 128]))

            # EPT[a, (c,s)] = F_c(s, a)
            EPT_p = ps.tile([A, 128], F32, tag="ps_ept")
            nc.tensor.matmul(EPT_p[:], EMe[:], D128[:], start=True, stop=True)
            # ln F1 table for the R1 term (reads PSUM directly)
            LF1 = sb.tile([A, S], F32, tag="LF1")
            nc.scalar.activation(LF1[:], EPT_p[:, 64:64 + S], AF.Ln)
            # 1/F1 -> SBUF (critical for W) before the F0 copy
            H1T = sb.tile([A, S], F32, tag="H1T")
            nc.vector.reciprocal(H1T[:], EPT_p[:, 64:64 + S])
            G0 = sb.tile([A, S], F32, tag="G0")
            nc.vector.tensor_copy(G0[:], EPT_p[:, 0:S])
            # priors (off the critical path; keep the scalar queue clear
            # until EPT/LF1 are through)
            P0e, ZP0 = rexp(PR0v, nmP0, 1, S, "P0")
            P1e, ZP1 = rexp(PR1v, nmP1, 1, S, "P1")
            lz0 = sb.tile([1, 1], F32, tag="lz0")
            nc.scalar.activation(lz0[:], ZP0[:], AF.Ln)
            lz1 = sb.tile([1, 1], F32, tag="lz1")
            nc.scalar.activation(lz1[:], ZP1[:], AF.Ln)

            # cnt rounded to fp32r for the weighted-sum matmuls
            CNTR = sb.tile([A, 1], F32R, tag="CNTR")
            nc.gpsimd.tensor_copy(CNTR[:], CNTv)

            # ---- big phase: W[a, s0, s1] = F0[s0,a] / F1[s1,a] ----
            W = sb.tile([A, S, S], F32, tag="W")
            SPX = sb.tile([A, N2], F32R, tag="SPX")
            NCH = 3
            SCH = S // NCH
            for j in range(NCH):
                lo = j * SCH
                nc.vector.tensor_mul(
                    W[:, lo:lo + SCH, :],
                    G0[:, lo:lo + SCH].unsqueeze(2)
                      .broadcast_to([A, SCH, S]),
                    H1T[:].unsqueeze(1).broadcast_to([A, SCH, S]))
                nc.scalar.activation(
                    SPX[:, lo * S:(lo + SCH) * S],
                    W[:, lo:lo + SCH, :].rearrange("p a b -> p (a b)"),
                    AF.Ln, bias=1.0)

            # NL row: cnt-weighted column sums; 3 rotating psum banks,
            # copies split vector-first so the PE never waits
            NLS = sb.tile([1, N2], F32, tag="NLS")
            NL_p0 = ps.tile([1, 512], F32, tag="ps_nla")
            NL_p1 = ps.tile([1, 512], F32, tag="ps_nlb")
            NL_p2 = ps.tile([1, 512], F32, tag="ps_nlc")
            banks = [NL_p0, NL_p1, NL_p2]

            def nl_mm(c):
                lo = 512 * c
                w = min(512, N2 - lo)
                nc.tensor.matmul(banks[c % 3][:, 0:w], CNTR[:],
                                 SPX[:, lo:lo + w], start=True, stop=True)

            nl_mm(0)
            nl_mm(1)
            nl_mm(2)
            nc.vector.tensor_copy(NLS[:, 0:512], NL_p0[:])
            nl_mm(3)
            nc.vector.tensor_copy(NLS[:, 512:1024], NL_p1[:])
            nl_mm(4)
            nc.vector.tensor_copy(NLS[:, 1024:1536], NL_p2[:])
            nc.scalar.copy(NLS[:, 1536:2048], NL_p0[:])
            nc.scalar.copy(NLS[:, 2048:2304], NL_p1[:, 0:256])

            # ---- bias grid B[s0,s1] = p0[s0] + p1[s1] + R1[s1] ----
            def prow(src, nm, lz, name):
                row = sb.tile([1, S], F32, tag=f"row{name}")
                nc.vector.scalar_tensor_tensor(
                    row[:], src, nm[:], lz[:].broadcast_to([1, S]),
                    op0=OP.add, op1=OP.subtract)
                return row

            p0row = prow(PR0v, nmP0, lz0, "P0")
            p1row = prow(PR1v, nmP1, lz1, "P1")
            R1_p = ps.tile([1, S], F32, tag="ps_r1")
            nc.tensor.matmul(R1_p[:], CNTv, LF1[:], start=True, stop=True)
            q1 = sb.tile([1, S], F32, tag="q1")
            nc.vector.tensor_add(q1[:], p1row[:], R1_p[:])
            p0c_p = ps.tile([S, 1], F32, tag="ps_p0")
            nc.tensor.matmul(p0c_p[:], p0row[:], ONE1[:], start=True,
                             stop=True)
            p0c = sb.tile([S, 1], F32, tag="p0c")
            nc.vector.tensor_copy(p0c[:], p0c_p[:])
            B2d_p = ps.tile([S, S], F32, tag="ps_b2d")
            nc.tensor.matmul(B2d_p[:], ONES48[:], q1[:], start=True,
                             stop=True)
            B2s = sb.tile([S, S], F32, tag="B2s")
            nc.vector.tensor_scalar_add(B2s[:], B2d_p[:], p0c[:])
            B16 = sb.tile([16, 144], F32, tag="B16")
            nc.scalar.dma_start(B16[:], B2s[:])

            NL16 = sb.tile([16, 144], F32, tag="NL16")
            nc.sync.dma_start(NL16[0:10, :], NLS[:, 0:1440])
            nc.sync.dma_start(NL16[10:16, :], NLS[:, 1440:N2])

            # ---- tail: LSE over beta = NL + B on 16 partitions ----
            T16 = sb.tile([16, 144], F32, tag="T16")
            nc.vector.tensor_add(T16[:], NL16[:], B16[:])
            M1 = sb.tile([16, 1], F32, tag="M1")
            nc.vector.tensor_reduce(M1[:], T16[:], axis=AX.X, op=OP.max)
            Mg = sb.tile([16, 1], F32, tag="Mg")
            nc.gpsimd.partition_all_reduce(Mg[:], M1[:], 16, ReduceOp.max)
            Mn = sb.tile([16, 1], F32, tag="Mn")
            nc.vector.tensor_scalar_mul(Mn[:], Mg[:], -1.0)
            EX16 = sb.tile([16, 144], F32, tag="EX16")
            S1 = sb.tile([16, 1], F32, tag="S1")
            nc.scalar.activation(EX16[:], T16[:], AF.Exp, bias=Mn[:],
                                 accum_out=S1[:])
            O16 = sb.tile([16, 1], F32, tag="O16")
            nc.vector.memset(O16[:], 1.0)
            Sg_p = ps.tile([1, 1], F32, tag="ps_r1")
            nc.tensor.matmul(Sg_p[:], S1[:], O16[:], start=True, stop=True)
            lnS = sb.tile([1, 1], F32, tag="lnS")
            nc.scalar.activation(lnS[:], Sg_p[:], AF.Ln)
            ans = sb.tile([1, 1], F32, tag="ans")
            nc.vector.tensor_add(ans[:], lnS[:], Mg[0:1, :])
            nc.sync.dma_start(out_d[:, :], ans[:])
            if dbg:
                EPTs = sb.tile([A, 128], F32, tag="EPTs")
                nc.vector.tensor_copy(EPTs[:], EPT_p[:])
                nc.sync.dma_start(dbg_ept[:, :], EPTs[:])
                nc.sync.dma_start(dbg_h1[:, :], H1T[:])
                nc.sync.dma_start(dbg_lf1[:, :], LF1[:])
                nc.sync.dma_start(dbg_b2s[:, :], B2s[:])
                nc.sync.dma_start(dbg_nls[:, :], NLS[:])
                nc.sync.dma_start(dbg_cd[:, :], cd[:])

    nc.compile()
    return nc


def _host_inputs(ys, transition, emission, choice, prior):
    ys = np.asarray(ys).astype(np.int64)
    packed = np.zeros((128, PACKW), np.float32)
    tr = np.asarray(transition, np.float32)
    em = np.asarray(emission, np.float32)
    pri = np.asarray(prior, np.float32)
    for c in range(K):
        r = c * 64
        packed[r:r + S, C_TR:C_TR + S] = tr[c]
        packed[r:r + S, C_EM:C_EM + A] = em[c]
        # pads: exp(NEG - max) == 0 keeps row sums finite
        packed[r + S:r + 64, C_TR:C_TR + S] = NEG
        packed[r + S:r + 64, C_EM:C_EM + A] = NEG
        for s in range(S):
            packed[r + s, C_DM + s] = 1.0       # trans-diag extract mask
    packed[0, C_P0:C_P0 + S] = pri[0]
    packed[0, C_P1:C_P1 + S] = pri[1]
    packed[0, C_CH:C_CH + K] = np.asarray(choice, np.float32)
    packed[0:A, C_CNT] = np.bincount(ys, minlength=A).astype(np.float32)
    imask = np.zeros((128, 128), np.float32)
    for c in range(K):
        for s in range(S):
            imask[c * 64 + s, c * 64 + s] = 1.0
    import jax.numpy as jnp
    return {"packed": packed, "imask": imask.astype(jnp.bfloat16)}


def kernel(ys, transition, emission, choice, prior):
    global _CACHED_NC
    if _CACHED_NC is None:
        _CACHED_NC = _build_nc()
    in_map = _host_inputs(ys, transition, emission, choice, prior)
    in_maps = [dict(in_map) for _ in range(N_CORES)]
    res = run_bass_kernel_spmd(_CACHED_NC, in_maps,
                               core_ids=list(range(N_CORES)))
    return np.float32(res.results[0]["out"][0, 0]).reshape(())


# revision 20
# speedup vs baseline: 1.2248x; 1.0291x over previous
"""Trainium2 Bass kernel for nn_InterleavedHiddenMarkovChain_47261820125822.

Math: in the reference, the dense (N,N) score matrix M (N = S*S*K = 4608)
is -inf except where the full state tuple of x_old equals x_new's (the
`same` mask compares all K components), so each column has exactly K=2
finite entries and the scan collapses exactly.  With probability-domain
tables F_c[s,a] = softmax(choice)[c] * softmax(trans[c,s,:])[s] *
softmax(emis[c,s,:])[a] and u_c = ln F_c:

    beta[s0,s1] = p0[s0] + p1[s1] + sum_t LSE(u0[s0,y_t], u1[s1,y_t])
    answer      = LSE_{s0,s1} beta

Using LSE(a,b) = b + log1p(exp(a-b)) and grouping the t-sum by symbol
counts cnt[a] = #{t: y_t = a} (integer prep on host):

    beta = p0[s0] + (p1 + sum_a cnt[a] u1[s1,a])[s1]
         + sum_a cnt[a] * log1p(F0[s0,a] / F1[s1,a])

so the only big work is one (A=64 part, S*S=2304 free) elementwise
multiply F0[a,s0] * (1/F1)[a,s1] (both free-dim broadcasts), one
log1p activation, and a cnt-weighted column-sum matmul (fp32r).

Perf structure (v2; baseline 65.7us -> v1 34.9us):
 - ALL inputs ride ONE packed DRAM image [128, 387] (one DMA) with pad
   rows prefilled; v1 spent ~6us issuing 11 DMAs + transfer stalls.
 - One manual InstLoadActFuncSet of the joint exp+ln table (id 6) ->
   exactly one activation-table load (the greedy inserter otherwise
   reloads on every exp<->ln switch; 13x1283ns in the baseline).
 - A dummy gpsimd custom op up front hoists the GPSIMD ucode library
   load off the critical path (v1 stalled ~2.5us before the first
   partition_broadcast).
 - softmaxes by division (vector.reciprocal + fused accum_out row
   sums); trans diagonal extracted on-device by mask-multiply-reduce.
 - cnt-weighted t-reduction as fp32r PE matmuls (4x fp32 rate) over 3
   rotating PSUM banks; copies split vector-first so the PE never
   waits on a bank.
 - bias grid B built by PE outer product, folded in a (16,144) layout;
   final LSE on 16 partitions via gpsimd partition_all_reduce (the
   baseline burned ~20us in (1,2304) single-lane ops here).

Sharding across the 8 cores: the collapsed problem is ~150K flops, far
below per-core fixed overheads, so the sharding-hint's row-sharded psum
scheme would be pure loss.  We replicate: all 8 cores run the identical
NEFF SPMD, and the host takes core 0's scalar.  Host does only integer
index prep (symbol counts, diag masks, layout packing); all float math
is on-device.
"""

import numpy as np

import concourse.bass as bass
import concourse.bacc as bacc
import concourse.mybir as mybir
from concourse import tile
from concourse.bass_isa import ReduceOp
from concourse.bass_utils import run_bass_kernel_spmd

F32 = mybir.dt.float32
F32R = mybir.dt.float32r
BF16 = mybir.dt.bfloat16
AF = mybir.ActivationFunctionType
AX = mybir.AxisListType
OP = mybir.AluOpType

K, S, A, T = 2, 48, 64, 64
N2 = S * S          # 2304
N_CORES = 8
NEG = -30.0         # pad fill; exp(NEG - max) == 0 to fp32
# packed input column layout
C_TR, C_EM, C_CH, C_CNT, C_DM, C_P0, C_P1 = (
    0, 48, 112, 114, 115, 163, 211)
PACKW = 259
C_CHUNK1 = C_DM
# act_info.json table index of natural_log_exp_and_others (exp + ln in
# one piecewise-poly table -> a single ACT_TABLE_LOAD serves the kernel)
ACT_TABLE_EXP_LN = 6

_CACHED_NC = None


def _build_nc(dbg=False):
    nc = bacc.Bacc("TRN2", target_bir_lowering=False, debug=False)

    pk = nc.dram_tensor("packed", [128, PACKW], F32, kind="ExternalInput")
    im_d = nc.dram_tensor("imask", [128, 128], BF16, kind="ExternalInput")
    out_d = nc.dram_tensor("out", [1, 1], F32, kind="ExternalOutput")
    if dbg:
        dbg_ept = nc.dram_tensor("dbg_ept", [A, 128], F32, kind="ExternalOutput")
        dbg_h1 = nc.dram_tensor("dbg_h1", [A, S], F32, kind="ExternalOutput")
        dbg_lf1 = nc.dram_tensor("dbg_lf1", [A, S], F32, kind="ExternalOutput")
        dbg_b2s = nc.dram_tensor("dbg_b2s", [S, S], F32, kind="ExternalOutput")
        dbg_nls = nc.dram_tensor("dbg_nls", [1, N2], F32, kind="ExternalOutput")
        dbg_cd = nc.dram_tensor("dbg_cd", [128, 1], F32, kind="ExternalOutput")

    with tile.TileContext(nc) as tc:
        with (
            tc.tile_pool(name="sb", bufs=1) as sb,
            tc.tile_pool(name="ps", bufs=1, space="PSUM") as ps,
        ):
            # activation table (exp+ln) + gpsimd ucode library warm-up;
            # both run while the input DMA is in flight
            nc.scalar.add_instruction(mybir.InstLoadActFuncSet(
                name=nc.get_next_instruction_name(),
                act_func_set_id=ACT_TABLE_EXP_LN, ins=[], outs=[]))
            with tc.high_priority():
                ONE1 = sb.tile([1, 1], F32, tag="ONE1")
                nc.vector.memset(ONE1[:], 1.0)
                DUM = sb.tile([2, 1], F32, tag="DUM")
                nc.gpsimd.partition_broadcast(DUM[:], ONE1[:])

            BIG = sb.tile([128, PACKW], F32, tag="BIG")
            IM = sb.tile([128, 128], BF16, tag="IM")
            nc.sync.dma_start(BIG[:, 0:C_CHUNK1], pk[:, 0:C_CHUNK1])
            nc.scalar.dma_start(BIG[:, C_CHUNK1:PACKW], pk[:, C_CHUNK1:PACKW])
            nc.gpsimd.dma_start(IM[:], im_d[:, :])
            TRXv = BIG[:, C_TR:C_TR + S]
            EMXv = BIG[:, C_EM:C_EM + A]
            DMv = BIG[:, C_DM:C_DM + S]
            PR0v = BIG[0:1, C_P0:C_P0 + S]
            PR1v = BIG[0:1, C_P1:C_P1 + S]
            CHXv = BIG[0:1, C_CH:C_CH + K]
            CNTv = BIG[0:A, C_CNT:C_CNT + 1]

            # constants
            ONES48 = sb.tile([1, S], F32, tag="ONES48")
            nc.vector.memset(ONES48[:], 1.0)
            O128 = sb.tile([1, 128], BF16, tag="O128")
            nc.vector.memset(O128[:], 1.0)
            cd = sb.tile([128, 1], F32, tag="cd")
            nc.vector.memset(cd[:], 0.0)

            # ---- row maxes (all only need BIG) ----
            def rmax(v, p, name):
                t = sb.tile([p, 1], F32, tag=f"nm{name}")
                nc.vector.tensor_reduce(t[:], v, axis=AX.X, op=OP.max,
                                        negate=True)
                return t

            nmE = rmax(EMXv, 128, "E")
            nmT = rmax(TRXv, 128, "T")
            nmC = rmax(CHXv, 1, "C")
            nmP0 = rmax(PR0v, 1, "P0")
            nmP1 = rmax(PR1v, 1, "P1")

            # ---- exps with fused row sums (scalar) ----
            def rexp(v, nm, p, w, name):
                e = sb.tile([p, w], F32, tag=f"e{name}")
                z = sb.tile([p, 1], F32, tag=f"z{name}")
                nc.scalar.activation(e[:], v, AF.Exp, bias=nm[:],
                                     accum_out=z[:])
                return e, z

            def rexp_na(v, nm, p, w, name, dt=F32):
                e = sb.tile([p, w], dt, tag=f"e{name}")
                nc.scalar.activation(e[:], v, AF.Exp, bias=nm[:])
                z = sb.tile([p, 1], F32, tag=f"z{name}")
                nc.vector.tensor_reduce(z[:], e[:], axis=AX.X, op=OP.add)
                return e, z

            EMe, ZE = rexp_na(EMXv, nmE, 128, A, "E", dt=BF16)
            TJ, ZT = rexp_na(TRXv, nmT, 128, S, "T")
            CHe, ZC = rexp_na(CHXv, nmC, 1, K, "C")

            # ---- vector normalize chain ----
            # emission row-normalizer 1/ZE folds into the diagonal scale
            ZEr = sb.tile([128, 1], F32, tag="ZEr")
            nc.vector.reciprocal(ZEr[:], ZE[:])
            ZTr0 = sb.tile([128, 1], F32, tag="ZTr0")
            nc.vector.reciprocal(ZTr0[:], ZT[:])
            ZTr = sb.tile([128, 1], F32, tag="ZTr")
            nc.vector.tensor_mul(ZTr[:], ZTr0[:], ZEr[:])
            # diagonal of exp(trans row): mask-multiply-reduce
            DJ = sb.tile([128, S], F32, tag="DJ")
            dex = sb.tile([128, 1], F32, tag="dex")
            nc.vector.scalar_tensor_tensor(DJ[:], TJ[:], 1.0, DMv,
                                           op0=OP.mult, op1=OP.mult,
                                           accum_out=dex[:])
            dpr = sb.tile([128, 1], F32, tag="dpr")
            nc.vector.tensor_mul(dpr[:], dex[:], ZTr[:])
            ZCr = sb.tile([1, 1], F32, tag="ZCr")
            nc.vector.reciprocal(ZCr[:], ZC[:])
            CPr = sb.tile([1, K], BF16, tag="CPr")
            nc.scalar.activation(CPr[:], CHe[:], AF.Copy, bias=0.0,
                                 scale=ZCr[:])

            # choice probs to all partitions via PE outer product
            CB_p = ps.tile([128, K], F32, tag="ps_cb")
            nc.tensor.matmul(CB_p[:], O128[:], CPr[:], start=True, stop=True)

            # cd[(c,s)] = c_prob[c] * d_prob[(c,s)]; scaled diagonal D128
            nc.vector.tensor_mul(cd[0:S, :], dpr[0:S, :], CB_p[0:S, 0:1])
            nc.vector.tensor_mul(cd[64:64 + S, :], dpr[64:64 + S, :],
                                 CB_p[64:64 + S, 1:2])
            D128 = sb.tile([128, 128], BF16, tag="D128")
            nc.vector.tensor_mul(D128[:], IM[:],
                                 cd[:].broadcast_to([128, 128]))

            # EPT[a, (c,s)] = F_c(s, a)
            EPT_p = ps.tile([A, 128], F32, tag="ps_ept")
            nc.tensor.matmul(EPT_p[:], EMe[:], D128[:], start=True, stop=True)
            # ln F1 table for the R1 term (reads PSUM directly)
            LF1 = sb.tile([A, S], F32, tag="LF1")
            nc.scalar.activation(LF1[:], EPT_p[:, 64:64 + S], AF.Ln)
            # 1/F1 -> SBUF (critical for W) before the F0 copy
            H1T = sb.tile([A, S], F32, tag="H1T")
            nc.vector.reciprocal(H1T[:], EPT_p[:, 64:64 + S])
            G0 = sb.tile([A, S], F32, tag="G0")
            nc.vector.tensor_copy(G0[:], EPT_p[:, 0:S])
            # priors (off the critical path; keep the scalar queue clear
            # until EPT/LF1 are through)
            P0e, ZP0 = rexp(PR0v, nmP0, 1, S, "P0")
            P1e, ZP1 = rexp(PR1v, nmP1, 1, S, "P1")
            lz0 = sb.tile([1, 1], F32, tag="lz0")
            nc.scalar.activation(lz0[:], ZP0[:], AF.Ln)
            lz1 = sb.tile([1, 1], F32, tag="lz1")
            nc.scalar.activation(lz1[:], ZP1[:], AF.Ln)

            # cnt rounded to fp32r for the weighted-sum matmuls
            with tc.high_priority():
                CNTR = sb.tile([A, 1], F32R, tag="CNTR")
                nc.gpsimd.tensor_copy(CNTR[:], CNTv)

            # ---- big phase: W[a, s0, s1] = F0[s0,a] / F1[s1,a] ----
            W = sb.tile([A, S, S], F32, tag="W")
            SPX = sb.tile([A, N2], F32R, tag="SPX")
            NCH = 3
            SCH = S // NCH
            for j in range(NCH):
                lo = j * SCH
                nc.vector.tensor_mul(
                    W[:, lo:lo + SCH, :],
                    G0[:, lo:lo + SCH].unsqueeze(2)
                      .broadcast_to([A, SCH, S]),
                    H1T[:].unsqueeze(1).broadcast_to([A, SCH, S]))
                nc.scalar.activation(
                    SPX[:, lo * S:(lo + SCH) * S],
                    W[:, lo:lo + SCH, :].rearrange("p a b -> p (a b)"),
                    AF.Ln, bias=1.0)

            # NL row: cnt-weighted column sums; 3 rotating psum banks,
            # copies split vector-first so the PE never waits
            NLS = sb.tile([1, N2], F32, tag="NLS")
            NL_p0 = ps.tile([1, 512], F32, tag="ps_nla")
            NL_p1 = ps.tile([1, 512], F32, tag="ps_nlb")
            NL_p2 = ps.tile([1, 512], F32, tag="ps_nlc")
            banks = [NL_p0, NL_p1, NL_p2]

            def nl_mm(c):
                lo = 512 * c
                w = min(512, N2 - lo)
                nc.tensor.matmul(banks[c % 3][:, 0:w], CNTR[:],
                                 SPX[:, lo:lo + w], start=True, stop=True)

            nl_mm(0)
            nl_mm(1)
            nl_mm(2)
            nc.vector.tensor_copy(NLS[:, 0:512], NL_p0[:])
            nl_mm(3)
            nc.vector.tensor_copy(NLS[:, 512:1024], NL_p1[:])
            nl_mm(4)
            nc.vector.tensor_copy(NLS[:, 1024:1536], NL_p2[:])
            nc.scalar.copy(NLS[:, 1536:2048], NL_p0[:])
            nc.scalar.copy(NLS[:, 2048:2304], NL_p1[:, 0:256])

            # ---- bias grid B[s0,s1] = p0[s0] + p1[s1] + R1[s1] ----
            def prow(src, nm, lz, name):
                row = sb.tile([1, S], F32, tag=f"row{name}")
                nc.vector.scalar_tensor_tensor(
                    row[:], src, nm[:], lz[:].broadcast_to([1, S]),
                    op0=OP.add, op1=OP.subtract)
                return row

            p0row = prow(PR0v, nmP0, lz0, "P0")
            p1row = prow(PR1v, nmP1, lz1, "P1")
            R1_p = ps.tile([1, S], F32, tag="ps_r1")
            nc.tensor.matmul(R1_p[:], CNTv, LF1[:], start=True, stop=True)
            q1 = sb.tile([1, S], F32, tag="q1")
            nc.vector.tensor_add(q1[:], p1row[:], R1_p[:])
            p0c_p = ps.tile([S, 1], F32, tag="ps_p0")
            nc.tensor.matmul(p0c_p[:], p0row[:], ONE1[:], start=True,
                             stop=True)
            p0c = sb.tile([S, 1], F32, tag="p0c")
            nc.vector.tensor_copy(p0c[:], p0c_p[:])
            B2d_p = ps.tile([S, S], F32, tag="ps_b2d")
            nc.tensor.matmul(B2d_p[:], ONES48[:], q1[:], start=True,
                             stop=True)
            B2s = sb.tile([S, S], F32, tag="B2s")
            nc.vector.tensor_scalar_add(B2s[:], B2d_p[:], p0c[:])
            B16 = sb.tile([16, 144], F32, tag="B16")
            nc.scalar.dma_start(B16[:], B2s[:])

            NL16 = sb.tile([16, 144], F32, tag="NL16")
            nc.sync.dma_start(NL16[0:10, :], NLS[:, 0:1440])
            nc.sync.dma_start(NL16[10:16, :], NLS[:, 1440:N2])

            # ---- tail: LSE over beta = NL + B on 16 partitions ----
            T16 = sb.tile([16, 144], F32, tag="T16")
            nc.vector.tensor_add(T16[:], NL16[:], B16[:])
            M1 = sb.tile([16, 1], F32, tag="M1")
            nc.vector.tensor_reduce(M1[:], T16[:], axis=AX.X, op=OP.max)
            Mg = sb.tile([16, 1], F32, tag="Mg")
            nc.gpsimd.partition_all_reduce(Mg[:], M1[:], 16, ReduceOp.max)
            Mn = sb.tile([16, 1], F32, tag="Mn")
            nc.vector.tensor_scalar_mul(Mn[:], Mg[:], -1.0)
            EX16 = sb.tile([16, 144], F32, tag="EX16")
            S1 = sb.tile([16, 1], F32, tag="S1")
            nc.scalar.activation(EX16[:], T16[:], AF.Exp, bias=Mn[:],
                                 accum_out=S1[:])
            O16 = sb.tile([16, 1], F32, tag="O16")
            nc.vector.memset(O16[:], 1.0)
            Sg_p = ps.tile([1, 1], F32, tag="ps_r1")
            nc.tensor.matmul(Sg_p[:], S1[:], O16[:], start=True, stop=True)
            lnS = sb.tile([1, 1], F32, tag="lnS")
            nc.scalar.activation(lnS[:], Sg_p[:], AF.Ln)
            ans = sb.tile([1, 1], F32, tag="ans")
            nc.vector.tensor_add(ans[:], lnS[:], Mg[0:1, :])
            nc.sync.dma_start(out_d[:, :], ans[:])
            if dbg:
                EPTs = sb.tile([A, 128], F32, tag="EPTs")
                nc.vector.tensor_copy(EPTs[:], EPT_p[:])
                nc.sync.dma_start(dbg_ept[:, :], EPTs[:])
                nc.sync.dma_start(dbg_h1[:, :], H1T[:])
                nc.sync.dma_start(dbg_lf1[:, :], LF1[:])
                nc.sync.dma_start(dbg_b2s[:, :], B2s[:])
                nc.sync.dma_start(dbg_nls[:, :], NLS[:])
                nc.sync.dma_start(dbg_cd[:, :], cd[:])

    nc.compile()
    return nc


def _host_inputs(ys, transition, emission, choice, prior):
    ys = np.asarray(ys).astype(np.int64)
    packed = np.zeros((128, PACKW), np.float32)
    tr = np.asarray(transition, np.float32)
    em = np.asarray(emission, np.float32)
    pri = np.asarray(prior, np.float32)
    for c in range(K):
        r = c * 64
        packed[r:r + S, C_TR:C_TR + S] = tr[c]
        packed[r:r + S, C_EM:C_EM + A] = em[c]
        # pads: exp(NEG - max) == 0 keeps row sums finite
        packed[r + S:r + 64, C_TR:C_TR + S] = NEG
        packed[r + S:r + 64, C_EM:C_EM + A] = NEG
        for s in range(S):
            packed[r + s, C_DM + s] = 1.0       # trans-diag extract mask
    packed[0, C_P0:C_P0 + S] = pri[0]
    packed[0, C_P1:C_P1 + S] = pri[1]
    packed[0, C_CH:C_CH + K] = np.asarray(choice, np.float32)
    packed[0:A, C_CNT] = np.bincount(ys, minlength=A).astype(np.float32)
    imask = np.zeros((128, 128), np.float32)
    for c in range(K):
        for s in range(S):
            imask[c * 64 + s, c * 64 + s] = 1.0
    import jax.numpy as jnp
    return {"packed": packed, "imask": imask.astype(jnp.bfloat16)}


def kernel(ys, transition, emission, choice, prior):
    global _CACHED_NC
    if _CACHED_NC is None:
        _CACHED_NC = _build_nc()
    in_map = _host_inputs(ys, transition, emission, choice, prior)
    in_maps = [dict(in_map) for _ in range(N_CORES)]
    res = run_bass_kernel_spmd(_CACHED_NC, in_maps,
                               core_ids=list(range(N_CORES)))
    return np.float32(res.results[0]["out"][0, 0]).reshape(())
